# revision 6
# baseline (speedup 1.0000x reference)
"""Trainium2 Bass kernel for the tanh-RNN problem (v5: inline output EMA).

Reference:
    xproj_t = input_t @ wi + brec
    z_t     = h_{t-1} @ wrec.T + xproj_t          (h_{-1} = h0)
    h_t     = 0.5 h_{t-1} + 0.5 tanh(z_t)
    out_t   = h_t @ wo

Structure:
  * z-form recurrence  z_{t+1} = 0.5 z_t + r_t @ (0.5 wrec.T) + (x_{t+1} - 0.5 x_t)
    with r_t = tanh(z_t); the 0.5 z_t term is injected into PSUM by a matmul
    with lhsT = 0.5*I reading an fp16 SBUF copy of z (one strided DVE CAST).
  * Time split into 16 contracting segments (W=48 warmup, ~5e-3 rel err
    verified end-to-end in simulation); 2 segments per core as interleaved
    streams so one stream's PSUM->ACT->PE tanh latency hides under the other
    stream's matmuls.  All 64 batch rows ride in every matmul.
  * The output projection commutes with the h-blend:
        out_t = 0.5 out_{t-1} + r_t @ (wo/2)   (+ seed term that decays 2^-t)
    so out is accumulated INLINE in PSUM (1 inject + 4 k-matmuls per step,
    skewed one step behind the recurrence so r_t is ready), with the EMA
    started 16 steps before the segment's output window (truncation 2^-16).
    This removes both the per-step G-history blend (DVE was the clock) and
    the whole output-projection tail.
  * PSUM: 4 banks hold the x'-preload (chunks of 4 steps, double buffered by
    step parity), 4 banks are out-accumulators (stream x parity ping-pong).
"""

import numpy as np

import concourse.bacc as bacc
import concourse.mybir as mybir
from concourse.tile import TileContext, add_dep_helper
from concourse import bass_utils

F16 = mybir.dt.float16
F32 = mybir.dt.float32

B, T_FULL, I, H, O = 64, 1024, 64, 512, 64
NCORES = 8
NST = 2                    # streams (time segments) per core
SEG = NCORES * NST         # 16 segments
SOUT = T_FULL // SEG       # 64 output steps per segment
W = 48                     # warmup steps per segment
TL = W + SOUT              # 112 local steps per stream
KT = H // 128              # 4 tiles over H
CH = 4                     # steps per psum refill chunk
NCH = TL // CH             # 28 chunks
NX = 3                     # extra x rows: brec, u=h0@wrec.T, a=arctanh(h0)
IR = I + NX                # 67 rhs rows for the x-projection
GB0 = W - 16               # first step entering the output EMA

# packed-weights column offsets (fp16, [128, WPK])
WT_OFF = 0                 # 4 k-tiles x 512
HI_OFF = 2048              # 0.5*I, 128
WO_OFF = 2176              # 4 k-tiles x 64 (wo/2)
WI_OFF = 2432              # wiA on rows 0:67, 512
WPK = 2944


def build():
    nc = bacc.Bacc("TRN2", target_bir_lowering=False, debug=False)
    pe_prev = [None]

    def mm(*args, **kw):
        inst = nc.tensor.matmul(*args, **kw)
        if pe_prev[0] is not None:
            add_dep_helper(inst.ins, pe_prev[0].ins, sync=False, reason="pe order")
        pe_prev[0] = inst
        return inst

    d_wpk = nc.dram_tensor("wpk", [128, WPK], F16, kind="ExternalInput")
    d_xT = nc.dram_tensor("xT", [IR, NST * TL * 64], F16, kind="ExternalInput")
    d_out = nc.dram_tensor("outT", [O, NST * SOUT * 64], F16, kind="ExternalOutput")

    with TileContext(nc) as tc:
        with (
            tc.tile_pool(name="wpool", bufs=1) as wpool,
            tc.tile_pool(name="rz", bufs=1) as rzpool,
            tc.tile_pool(name="px", bufs=1, space="PSUM") as px,
        ):
            wpk = wpool.tile([128, WPK], F16, tag="wpk")
            nc.sync.dma_start(wpk[:], d_wpk[:])

            xT = wpool.tile([IR, NST * TL * 64], F16, tag="xT")
            CB = NST * CH * 64                       # cols per chunk = 512
            for lo, hi in ((0, 4 * CB), (4 * CB, 16 * CB), (16 * CB, NCH * CB)):
                nc.sync.dma_start(xT[:, lo:hi], d_xT[:, lo:hi])

            wT = [wpk[:, WT_OFF + k * 512 : WT_OFF + (k + 1) * 512] for k in range(KT)]
            hI = wpk[:, HI_OFF : HI_OFF + 128]
            hI64 = wpk[0:64, HI_OFF : HI_OFF + 64]
            wo = [wpk[:, WO_OFF + k * 64 : WO_OFF + (k + 1) * 64] for k in range(KT)]
            wi = wpk[:, WI_OFF : WI_OFF + 512]

            r_t = [[rzpool.tile([128, KT * 64], F16, tag=f"r{st}{p}", name=f"r{st}{p}")
                    for p in range(2)] for st in range(NST)]
            zsb = [[rzpool.tile([128, KT * 64], F16, tag=f"z{st}{p}", name=f"z{st}{p}")
                    for p in range(2)] for st in range(NST)]
            # fp16 staging for the output EMA (+ scratch cols for warmup steps)
            ostg = [wpool.tile([64, SOUT * 64 + 128], F16, tag=f"os{st}", name=f"os{st}")
                    for st in range(NST)]

            # psum: 4 x-banks [4m x 2q x 64b] + 4 out-accumulator banks
            bank = [[px.tile([128, 512], F32, tag=f"px{st}{p}", name=f"px{st}{p}")
                     for p in range(2)] for st in range(NST)]
            obank = [[px.tile([128, 512], F32, tag=f"po{st}{p}", name=f"po{st}{p}")
                      for p in range(2)] for st in range(NST)]

            xTr = xT.rearrange(
                "p (c s par q b) -> p c s par q b",
                c=NCH, s=NST, par=2, q=CH // 2, b=64,
            )

            def refill(st, c, par):
                rhs = xTr[:, c, st, par, :, :]          # [IR, 2, 64] contiguous
                for m in range(KT):
                    mm(
                        bank[st][par][:, m * 128 : (m + 1) * 128],
                        lhsT=wi[:IR, m * 128 : (m + 1) * 128],
                        rhs=rhs,
                        start=(m == 0),
                        stop=False,
                        skip_group_check=True,
                    )

            def ostg_ap(st, t):
                """fp16 staging slot for out_t (scratch cols during warmup)."""
                if t >= W:
                    return ostg[st][:, (t - W) * 64 : (t - W + 1) * 64]
                return ostg[st][:, SOUT * 64 + (t % 2) * 64 : SOUT * 64 + (t % 2 + 1) * 64]

            for st in range(NST):
                refill(st, 0, 0)
                refill(st, 0, 1)

            for t in range(TL):
                c, tt = divmod(t, CH)
                par, q = tt % 2, tt // 2
                for st in range(NST):
                    if tt == CH - 1 and c + 1 < NCH:
                        refill(st, c + 1, 0)
                    if tt == 0 and c > 0:
                        refill(st, c, 1)
                    bq = bank[st][par].rearrange("p (m c) -> p m c", c=128)[
                        :, :, q * 64 : (q + 1) * 64
                    ]                                     # [128, 4m, 64] this step
                    if t > 0:
                        mm(                               # inject 0.5*z_{t-1}
                            bq[:],
                            lhsT=hI,
                            rhs=zsb[st][1 - par].rearrange("p (m b) -> p m b", b=64)[:],
                            start=False,
                            stop=False,
                            skip_group_check=True,
                        )
                        for k in range(KT):
                            for m in range(KT):
                                mm(
                                    bank[st][par][:, m * 128 + q * 64 : m * 128 + (q + 1) * 64],
                                    lhsT=wT[k][:, m * 128 : (m + 1) * 128],
                                    rhs=r_t[st][1 - par][:, k * 64 : (k + 1) * 64],
                                    start=False,
                                    stop=False,
                                    skip_group_check=True,
                                )
                    # skewed output EMA for step t-1 (its r is ready by now):
                    # out_{t-1} = 0.5 out_{t-2} + r_{t-1} @ (wo/2)
                    tp = t - 1
                    if tp >= GB0:
                        po = obank[st][tp % 2][:O, 0:64]
                        if tp > GB0:
                            mm(po, lhsT=hI64, rhs=ostg_ap(st, tp - 1),
                               start=True, stop=False, skip_group_check=True)
                        for k in range(KT):
                            mm(po, lhsT=wo[k],
                               rhs=r_t[st][1 - par][:, k * 64 : (k + 1) * 64],
                               start=(k == 0 and tp == GB0),
                               stop=(k == KT - 1),
                               skip_group_check=True)
                        nc.vector.tensor_copy(ostg_ap(st, tp), po)
                        u = tp - W
                        if tp >= W and u % 8 == 7:        # flush 8 finished cols
                            nc.sync.dma_start(
                                d_out[:, st * SOUT * 64 + (u - 7) * 64 :
                                      st * SOUT * 64 + (u + 1) * 64],
                                ostg[st][:, (u - 7) * 64 : (u + 1) * 64],
                            )
                    rv = r_t[st][par].rearrange("p (m b) -> p m b", b=64)
                    nc.scalar.activation(rv[:], bq, mybir.ActivationFunctionType.Tanh)
                    nc.vector.tensor_copy(
                        zsb[st][par].rearrange("p (m b) -> p m b", b=64)[:], bq
                    )

            # drain: output EMA for the final step of each stream
            for st in range(NST):
                tp = TL - 1
                par = tp % 2                              # parity of step tp
                po = obank[st][tp % 2][:O, 0:64]
                mm(po, lhsT=hI64, rhs=ostg_ap(st, tp - 1),
                   start=True, stop=False, skip_group_check=True)
                for k in range(KT):
                    mm(po, lhsT=wo[k], rhs=r_t[st][par][:, k * 64 : (k + 1) * 64],
                       start=False, stop=(k == KT - 1), skip_group_check=True)
                nc.vector.tensor_copy(ostg_ap(st, tp), po)
                u = tp - W
                nc.sync.dma_start(
                    d_out[:, st * SOUT * 64 + (u - 7) * 64 : st * SOUT * 64 + (u + 1) * 64],
                    ostg[st][:, (u - 7) * 64 : (u + 1) * 64],
                )

    _thin_pe_clock(nc)
    nc.compile()
    return nc


def _thin_pe_clock(nc):
    """Strip unreferenced PE engine-clock increments from the BIR.

    Tile attaches a sem-inc to EVERY matmul; the semaphore-update pipeline
    sustains only ~34ns/inc, so the inc stream (not the PE) can become the
    clock.  Keeping increments only at ticks some wait references (and
    remapping waits to their rank) is semantically equivalent.
    """
    import bisect

    fn = nc.m.functions[0]
    SEM = None
    for blk in fn.blocks:
        for inst in blk.instructions:
            si = inst.sync_info
            if si is None:
                continue
            for u in si.on_update:
                if u.ant_name and u.ant_name.startswith("PE_") and u.update_mode == "sem-inc":
                    SEM = u.id
                    break
            if SEM is not None:
                break
        if SEM is not None:
            break
    if SEM is None:
        return
    refs = set()
    for blk in fn.blocks:
        for inst in blk.instructions:
            si = inst.sync_info
            if si is None:
                continue
            for w in si.on_wait:
                if w.id == SEM:
                    assert w.wait_mode == "sem-ge-imm", w.wait_mode
                    refs.add(w.wait_value)
    kept = sorted(refs)
    tick = 0
    for blk in fn.blocks:
        for inst in blk.instructions:
            si = inst.sync_info
            if si is None:
                continue
            ups = list(si.on_update)
            has = [u for u in ups if u.id == SEM]
            if has:
                assert len(has) == 1 and has[0].update_value == 1
                tick += 1
                if tick not in refs:
                    si.on_update = [u for u in ups if u.id != SEM]
    for blk in fn.blocks:
        for inst in blk.instructions:
            si = inst.sync_info
            if si is None:
                continue
            for w in si.on_wait:
                if w.id == SEM:
                    w.wait_value = bisect.bisect_right(kept, w.wait_value)


_CACHE = {}


def _get_nc():
    if "nc" not in _CACHE:
        _CACHE["nc"] = build()
    return _CACHE["nc"]


def prep_inputs(input, wi, wrec, wo, brec, h0):
    """Host-side layout prep. Returns list of 8 in_maps (xT differs per core)."""
    input = np.asarray(input, dtype=np.float32)
    wi = np.asarray(wi, dtype=np.float32)
    wrec = np.asarray(wrec, dtype=np.float32)
    wo = np.asarray(wo, dtype=np.float32)
    brec = np.asarray(brec, dtype=np.float32)
    h0 = np.asarray(h0, dtype=np.float32)

    wTh = (0.5 * wrec.T).astype(np.float16)
    h0c = np.clip(h0, -1 + 1e-6, 1 - 1e-6)
    a_vec = np.arctanh(h0c).astype(np.float32)
    u_vec = 2.0 * (h0c @ wTh.astype(np.float32))     # h0 @ wrec.T (quantized)
    wiA = np.concatenate(
        [wi, brec[None, :], u_vec[None, :], a_vec[None, :]], axis=0
    ).astype(np.float16)

    wpk = np.zeros((128, WPK), np.float16)
    for k in range(KT):
        wpk[:, WT_OFF + k * 512 : WT_OFF + (k + 1) * 512] = wTh[k * 128 : (k + 1) * 128]
    wpk[:, HI_OFF : HI_OFF + 128] = (0.5 * np.eye(128)).astype(np.float16)
    woh = (wo / 2.0).astype(np.float16)
    for k in range(KT):
        wpk[:, WO_OFF + k * 64 : WO_OFF + (k + 1) * 64] = woh[k * 128 : (k + 1) * 128]
    wpk[:IR, WI_OFF : WI_OFF + 512] = wiA

    x16 = input.astype(np.float16).astype(np.float32)

    in_maps = []
    for core in range(NCORES):
        xA = np.zeros((IR, NST, TL, 64), np.float32)
        for st in range(NST):
            s = NST * core + st
            t0 = s * SOUT
            for j in range(TL):
                g = t0 - W + j
                if s == 0:
                    if j == 0:
                        xA[I + 2, st, j] = 1.0                       # z0 = arctanh(h0)
                    elif j < W:
                        xA[I + 1, st, j] = -0.5                      # hold z at z*
                        xA[I + 2, st, j] = 0.5
                    elif j == W:
                        xA[:I, st, j] = x16[:, 0].T                  # onto true x_0
                        xA[I, st, j] = 1.0
                        xA[I + 1, st, j] = 0.5
                        xA[I + 2, st, j] = -0.5
                    else:
                        xA[:I, st, j] = (x16[:, g] - 0.5 * x16[:, g - 1]).T
                        xA[I, st, j] = 0.5
                else:
                    if j == 0:
                        xA[:I, st, j] = x16[:, g].T                  # z0 = h0 wrec.T + x
                        xA[I, st, j] = 1.0
                        xA[I + 1, st, j] = 1.0
                    else:
                        xA[:I, st, j] = (x16[:, g] - 0.5 * x16[:, g - 1]).T
                        xA[I, st, j] = 0.5
        # chunk-major reorder: [st, (c,q,par)] -> [c, st, par, q]
        xA = xA.reshape(IR, NST, NCH, CH // 2, 2, 64).transpose(0, 2, 1, 4, 3, 5)
        xA = np.ascontiguousarray(xA).reshape(IR, NST * TL * 64).astype(np.float16)
        in_maps.append({"wpk": wpk, "xT": xA})
    return in_maps


def run_sharded(inputs, t_steps=T_FULL, trace=False):
    assert t_steps == T_FULL, "kernel is built for the full 1024 steps"
    nc = _get_nc()
    in_maps = prep_inputs(**inputs)
    res = bass_utils.run_bass_kernel_spmd(
        nc, in_maps, core_ids=list(range(NCORES)), trace=trace
    )
    out = np.empty((B, T_FULL, O), np.float32)
    for core in range(NCORES):
        oT = res.results[core]["outT"].astype(np.float32)  # [O, NST*SOUT*64]
        for st in range(NST):
            s = NST * core + st
            blk = oT[:, st * SOUT * 64 : (st + 1) * SOUT * 64].reshape(O, SOUT, 64)
            out[:, s * SOUT : (s + 1) * SOUT] = np.transpose(blk, (2, 1, 0))
    return out, res


def kernel(input, wi, wrec, wo, brec, h0):
    out, _ = run_sharded(
        dict(input=input, wi=wi, wrec=wrec, wo=wo, brec=brec, h0=h0),
        t_steps=T_FULL,
        trace=False,
    )
    return out


# revision 10
# speedup vs baseline: 1.0397x; 1.0397x over previous
"""Trainium2 Bass kernel for the tanh-RNN problem (v5: inline output EMA).

Reference:
    xproj_t = input_t @ wi + brec
    z_t     = h_{t-1} @ wrec.T + xproj_t          (h_{-1} = h0)
    h_t     = 0.5 h_{t-1} + 0.5 tanh(z_t)
    out_t   = h_t @ wo

Structure:
  * z-form recurrence  z_{t+1} = 0.5 z_t + r_t @ (0.5 wrec.T) + (x_{t+1} - 0.5 x_t)
    with r_t = tanh(z_t); the 0.5 z_t term is injected into PSUM by a matmul
    with lhsT = 0.5*I reading an fp16 SBUF copy of z (one strided DVE CAST).
  * Time split into 16 contracting segments (W=48 warmup, ~5e-3 rel err
    verified end-to-end in simulation); 2 segments per core as interleaved
    streams so one stream's PSUM->ACT->PE tanh latency hides under the other
    stream's matmuls.  All 64 batch rows ride in every matmul.
  * The output projection commutes with the h-blend:
        out_t = 0.5 out_{t-1} + r_t @ (wo/2)   (+ seed term that decays 2^-t)
    so out is accumulated INLINE in PSUM (1 inject + 4 k-matmuls per step,
    skewed one step behind the recurrence so r_t is ready), with the EMA
    started 16 steps before the segment's output window (truncation 2^-16).
    This removes both the per-step G-history blend (DVE was the clock) and
    the whole output-projection tail.
  * PSUM: 4 banks hold the x'-preload (chunks of 4 steps, double buffered by
    step parity), 4 banks are out-accumulators (stream x parity ping-pong).
"""

import numpy as np

import concourse.bacc as bacc
import concourse.mybir as mybir
from concourse.tile import TileContext, add_dep_helper
from concourse import bass_utils

F16 = mybir.dt.float16
F32 = mybir.dt.float32

B, T_FULL, I, H, O = 64, 1024, 64, 512, 64
NCORES = 8
NST = 2                    # streams (time segments) per core
SEG = NCORES * NST         # 16 segments
SOUT = T_FULL // SEG       # 64 output steps per segment
W = 40                     # warmup steps per segment
TL = W + SOUT              # 112 local steps per stream
KT = H // 128              # 4 tiles over H
CH = 4                     # steps per psum refill chunk
NCH = TL // CH             # 28 chunks
NX = 3                     # extra x rows: brec, u=h0@wrec.T, a=arctanh(h0)
IR = I + NX                # 67 rhs rows for the x-projection
GB0 = W - 16               # first step entering the output EMA

# packed-weights column offsets (fp16, [128, WPK])
WT_OFF = 0                 # 4 k-tiles x 512
HI_OFF = 2048              # 0.5*I, 128
WO_OFF = 2176              # 4 k-tiles x 64 (wo/2)
WI_OFF = 2432              # wiA on rows 0:67, 512
WPK = 2944


def build():
    nc = bacc.Bacc("TRN2", target_bir_lowering=False, debug=False)
    pe_prev = [None]

    def mm(*args, **kw):
        inst = nc.tensor.matmul(*args, **kw)
        if pe_prev[0] is not None:
            add_dep_helper(inst.ins, pe_prev[0].ins, sync=False, reason="pe order")
        pe_prev[0] = inst
        return inst

    d_wpk = nc.dram_tensor("wpk", [128, WPK], F16, kind="ExternalInput")
    d_xT = nc.dram_tensor("xT", [IR, NST * TL * 64], F16, kind="ExternalInput")
    d_out = nc.dram_tensor("outT", [O, NST * SOUT * 64], F16, kind="ExternalOutput")

    with TileContext(nc) as tc:
        with (
            tc.tile_pool(name="wpool", bufs=1) as wpool,
            tc.tile_pool(name="rz", bufs=1) as rzpool,
            tc.tile_pool(name="px", bufs=1, space="PSUM") as px,
        ):
            wpk = wpool.tile([128, WPK], F16, tag="wpk")
            nc.sync.dma_start(wpk[:], d_wpk[:])

            xT = wpool.tile([IR, NST * TL * 64], F16, tag="xT")
            CB = NST * CH * 64                       # cols per chunk = 512
            # x pieces go on the ACT DMA queue so they overlap the wpk DMA
            for lo, hi in ((0, 4 * CB), (4 * CB, 16 * CB), (16 * CB, NCH * CB)):
                nc.scalar.dma_start(xT[:, lo:hi], d_xT[:, lo:hi])

            wT = [wpk[:, WT_OFF + k * 512 : WT_OFF + (k + 1) * 512] for k in range(KT)]
            hI = wpk[:, HI_OFF : HI_OFF + 128]
            hI64 = wpk[0:64, HI_OFF : HI_OFF + 64]
            wo = [wpk[:, WO_OFF + k * 64 : WO_OFF + (k + 1) * 64] for k in range(KT)]
            wi = wpk[:, WI_OFF : WI_OFF + 512]

            r_t = [[rzpool.tile([128, KT * 64], F16, tag=f"r{st}{p}", name=f"r{st}{p}")
                    for p in range(2)] for st in range(NST)]
            zsb = [[rzpool.tile([128, KT * 64], F16, tag=f"z{st}{p}", name=f"z{st}{p}")
                    for p in range(2)] for st in range(NST)]
            # fp16 staging for the output EMA (+ scratch cols for warmup steps)
            ostg = [wpool.tile([64, SOUT * 64 + 128], F16, tag=f"os{st}", name=f"os{st}")
                    for st in range(NST)]

            # psum: 4 x-banks [4m x 2q x 64b] + 4 out-accumulator banks
            bank = [[px.tile([128, 512], F32, tag=f"px{st}{p}", name=f"px{st}{p}")
                     for p in range(2)] for st in range(NST)]
            obank = [[px.tile([128, 512], F32, tag=f"po{st}{p}", name=f"po{st}{p}")
                      for p in range(2)] for st in range(NST)]

            xTr = xT.rearrange(
                "p (c s par q b) -> p c s par q b",
                c=NCH, s=NST, par=2, q=CH // 2, b=64,
            )

            def refill(st, c, par):
                rhs = xTr[:, c, st, par, :, :]          # [IR, 2, 64] contiguous
                for m in range(KT):
                    mm(
                        bank[st][par][:, m * 128 : (m + 1) * 128],
                        lhsT=wi[:IR, m * 128 : (m + 1) * 128],
                        rhs=rhs,
                        start=(m == 0),
                        stop=False,
                        skip_group_check=True,
                    )

            def ostg_ap(st, t):
                """fp16 staging slot for out_t (scratch cols during warmup)."""
                if t >= W:
                    return ostg[st][:, (t - W) * 64 : (t - W + 1) * 64]
                return ostg[st][:, SOUT * 64 + (t % 2) * 64 : SOUT * 64 + (t % 2 + 1) * 64]

            for st in range(NST):
                refill(st, 0, 0)
                refill(st, 0, 1)

            for t in range(TL):
                c, tt = divmod(t, CH)
                par, q = tt % 2, tt // 2
                for st in range(NST):
                    if tt == CH - 1 and c + 1 < NCH:
                        refill(st, c + 1, 0)
                    if tt == 0 and c > 0:
                        refill(st, c, 1)
                    bq = bank[st][par].rearrange("p (m c) -> p m c", c=128)[
                        :, :, q * 64 : (q + 1) * 64
                    ]                                     # [128, 4m, 64] this step
                    # skewed output EMA for step t-1, FIRST in the slot (its
                    # inputs are a full slot old, so it absorbs chain latency
                    # and its psum result is ready early for the out-CAST):
                    # out_{t-1} = 0.5 out_{t-2} + r_{t-1} @ (wo/2)
                    tp = t - 1
                    if tp >= GB0:
                        po = obank[st][tp % 2][:O, 0:64]
                        if tp > GB0:
                            mm(po, lhsT=hI64, rhs=ostg_ap(st, tp - 1),
                               start=True, stop=False, skip_group_check=True)
                        for k in range(KT):
                            mm(po, lhsT=wo[k],
                               rhs=r_t[st][1 - par][:, k * 64 : (k + 1) * 64],
                               start=(k == 0 and tp == GB0),
                               stop=(k == KT - 1),
                               skip_group_check=True)
                        nc.vector.tensor_copy(ostg_ap(st, tp), po)
                        u = tp - W
                        if tp >= W and u % 8 == 7:        # flush 8 finished cols
                            nc.sync.dma_start(
                                d_out[:, st * SOUT * 64 + (u - 7) * 64 :
                                      st * SOUT * 64 + (u + 1) * 64],
                                ostg[st][:, (u - 7) * 64 : (u + 1) * 64],
                            )
                    if t > 0:
                        mm(                               # inject 0.5*z_{t-1}
                            bq[:],
                            lhsT=hI,
                            rhs=zsb[st][1 - par].rearrange("p (m b) -> p m b", b=64)[:],
                            start=False,
                            stop=False,
                            skip_group_check=True,
                        )
                        for k in range(KT):
                            for m in range(KT):
                                mm(
                                    bank[st][par][:, m * 128 + q * 64 : m * 128 + (q + 1) * 64],
                                    lhsT=wT[k][:, m * 128 : (m + 1) * 128],
                                    rhs=r_t[st][1 - par][:, k * 64 : (k + 1) * 64],
                                    start=False,
                                    stop=False,
                                    skip_group_check=True,
                                )
                    rv = r_t[st][par].rearrange("p (m b) -> p m b", b=64)
                    nc.scalar.activation(rv[:], bq, mybir.ActivationFunctionType.Tanh)
                    nc.vector.tensor_copy(
                        zsb[st][par].rearrange("p (m b) -> p m b", b=64)[:], bq
                    )

            # drain: output EMA for the final step of each stream
            for st in range(NST):
                tp = TL - 1
                par = tp % 2                              # parity of step tp
                po = obank[st][tp % 2][:O, 0:64]
                mm(po, lhsT=hI64, rhs=ostg_ap(st, tp - 1),
                   start=True, stop=False, skip_group_check=True)
                for k in range(KT):
                    mm(po, lhsT=wo[k], rhs=r_t[st][par][:, k * 64 : (k + 1) * 64],
                       start=False, stop=(k == KT - 1), skip_group_check=True)
                nc.vector.tensor_copy(ostg_ap(st, tp), po)
                u = tp - W
                nc.sync.dma_start(
                    d_out[:, st * SOUT * 64 + (u - 7) * 64 : st * SOUT * 64 + (u + 1) * 64],
                    ostg[st][:, (u - 7) * 64 : (u + 1) * 64],
                )

    _thin_pe_clock(nc)
    nc.compile()
    return nc


def _thin_pe_clock(nc):
    """Strip unreferenced PE engine-clock increments from the BIR.

    Tile attaches a sem-inc to EVERY matmul; the semaphore-update pipeline
    sustains only ~34ns/inc, so the inc stream (not the PE) can become the
    clock.  Keeping increments only at ticks some wait references (and
    remapping waits to their rank) is semantically equivalent.
    """
    import bisect

    fn = nc.m.functions[0]
    SEM = None
    for blk in fn.blocks:
        for inst in blk.instructions:
            si = inst.sync_info
            if si is None:
                continue
            for u in si.on_update:
                if u.ant_name and u.ant_name.startswith("PE_") and u.update_mode == "sem-inc":
                    SEM = u.id
                    break
            if SEM is not None:
                break
        if SEM is not None:
            break
    if SEM is None:
        return
    refs = set()
    for blk in fn.blocks:
        for inst in blk.instructions:
            si = inst.sync_info
            if si is None:
                continue
            for w in si.on_wait:
                if w.id == SEM:
                    assert w.wait_mode == "sem-ge-imm", w.wait_mode
                    refs.add(w.wait_value)
    kept = sorted(refs)
    tick = 0
    for blk in fn.blocks:
        for inst in blk.instructions:
            si = inst.sync_info
            if si is None:
                continue
            ups = list(si.on_update)
            has = [u for u in ups if u.id == SEM]
            if has:
                assert len(has) == 1 and has[0].update_value == 1
                tick += 1
                if tick not in refs:
                    si.on_update = [u for u in ups if u.id != SEM]
    for blk in fn.blocks:
        for inst in blk.instructions:
            si = inst.sync_info
            if si is None:
                continue
            for w in si.on_wait:
                if w.id == SEM:
                    w.wait_value = bisect.bisect_right(kept, w.wait_value)


_CACHE = {}


def _get_nc():
    if "nc" not in _CACHE:
        _CACHE["nc"] = build()
    return _CACHE["nc"]


def prep_inputs(input, wi, wrec, wo, brec, h0):
    """Host-side layout prep. Returns list of 8 in_maps (xT differs per core)."""
    input = np.asarray(input, dtype=np.float32)
    wi = np.asarray(wi, dtype=np.float32)
    wrec = np.asarray(wrec, dtype=np.float32)
    wo = np.asarray(wo, dtype=np.float32)
    brec = np.asarray(brec, dtype=np.float32)
    h0 = np.asarray(h0, dtype=np.float32)

    wTh = (0.5 * wrec.T).astype(np.float16)
    h0c = np.clip(h0, -1 + 1e-6, 1 - 1e-6)
    a_vec = np.arctanh(h0c).astype(np.float32)
    u_vec = 2.0 * (h0c @ wTh.astype(np.float32))     # h0 @ wrec.T (quantized)
    wiA = np.concatenate(
        [wi, brec[None, :], u_vec[None, :], a_vec[None, :]], axis=0
    ).astype(np.float16)

    wpk = np.zeros((128, WPK), np.float16)
    for k in range(KT):
        wpk[:, WT_OFF + k * 512 : WT_OFF + (k + 1) * 512] = wTh[k * 128 : (k + 1) * 128]
    wpk[:, HI_OFF : HI_OFF + 128] = (0.5 * np.eye(128)).astype(np.float16)
    woh = (wo / 2.0).astype(np.float16)
    for k in range(KT):
        wpk[:, WO_OFF + k * 64 : WO_OFF + (k + 1) * 64] = woh[k * 128 : (k + 1) * 128]
    wpk[:IR, WI_OFF : WI_OFF + 512] = wiA

    x16 = input.astype(np.float16).astype(np.float32)

    in_maps = []
    for core in range(NCORES):
        xA = np.zeros((IR, NST, TL, 64), np.float32)
        for st in range(NST):
            s = NST * core + st
            t0 = s * SOUT
            for j in range(TL):
                g = t0 - W + j
                if s == 0:
                    if j == 0:
                        xA[I + 2, st, j] = 1.0                       # z0 = arctanh(h0)
                    elif j < W:
                        xA[I + 1, st, j] = -0.5                      # hold z at z*
                        xA[I + 2, st, j] = 0.5
                    elif j == W:
                        xA[:I, st, j] = x16[:, 0].T                  # onto true x_0
                        xA[I, st, j] = 1.0
                        xA[I + 1, st, j] = 0.5
                        xA[I + 2, st, j] = -0.5
                    else:
                        xA[:I, st, j] = (x16[:, g] - 0.5 * x16[:, g - 1]).T
                        xA[I, st, j] = 0.5
                else:
                    if j == 0:
                        xA[:I, st, j] = x16[:, g].T                  # z0 = h0 wrec.T + x
                        xA[I, st, j] = 1.0
                        xA[I + 1, st, j] = 1.0
                    else:
                        xA[:I, st, j] = (x16[:, g] - 0.5 * x16[:, g - 1]).T
                        xA[I, st, j] = 0.5
        # chunk-major reorder: [st, (c,q,par)] -> [c, st, par, q]
        xA = xA.reshape(IR, NST, NCH, CH // 2, 2, 64).transpose(0, 2, 1, 4, 3, 5)
        xA = np.ascontiguousarray(xA).reshape(IR, NST * TL * 64).astype(np.float16)
        in_maps.append({"wpk": wpk, "xT": xA})
    return in_maps


def run_sharded(inputs, t_steps=T_FULL, trace=False):
    assert t_steps == T_FULL, "kernel is built for the full 1024 steps"
    nc = _get_nc()
    in_maps = prep_inputs(**inputs)
    res = bass_utils.run_bass_kernel_spmd(
        nc, in_maps, core_ids=list(range(NCORES)), trace=trace
    )
    out = np.empty((B, T_FULL, O), np.float32)
    for core in range(NCORES):
        oT = res.results[core]["outT"].astype(np.float32)  # [O, NST*SOUT*64]
        for st in range(NST):
            s = NST * core + st
            blk = oT[:, st * SOUT * 64 : (st + 1) * SOUT * 64].reshape(O, SOUT, 64)
            out[:, s * SOUT : (s + 1) * SOUT] = np.transpose(blk, (2, 1, 0))
    return out, res


def kernel(input, wi, wrec, wo, brec, h0):
    out, _ = run_sharded(
        dict(input=input, wi=wi, wrec=wrec, wo=wo, brec=brec, h0=h0),
        t_steps=T_FULL,
        trace=False,
    )
    return out


# revision 11
# speedup vs baseline: 1.1239x; 1.0810x over previous
"""Trainium2 Bass kernel for the tanh-RNN problem (v5: inline output EMA).

Reference:
    xproj_t = input_t @ wi + brec
    z_t     = h_{t-1} @ wrec.T + xproj_t          (h_{-1} = h0)
    h_t     = 0.5 h_{t-1} + 0.5 tanh(z_t)
    out_t   = h_t @ wo

Structure:
  * z-form recurrence  z_{t+1} = 0.5 z_t + r_t @ (0.5 wrec.T) + (x_{t+1} - 0.5 x_t)
    with r_t = tanh(z_t); the 0.5 z_t term is injected into PSUM by a matmul
    with lhsT = 0.5*I reading an fp16 SBUF copy of z (one strided DVE CAST).
  * Time split into 16 contracting segments (W=48 warmup, ~5e-3 rel err
    verified end-to-end in simulation); 2 segments per core as interleaved
    streams so one stream's PSUM->ACT->PE tanh latency hides under the other
    stream's matmuls.  All 64 batch rows ride in every matmul.
  * The output projection commutes with the h-blend:
        out_t = 0.5 out_{t-1} + r_t @ (wo/2)   (+ seed term that decays 2^-t)
    so out is accumulated INLINE in PSUM (1 inject + 4 k-matmuls per step,
    skewed one step behind the recurrence so r_t is ready), with the EMA
    started 16 steps before the segment's output window (truncation 2^-16).
    This removes both the per-step G-history blend (DVE was the clock) and
    the whole output-projection tail.
  * PSUM: 4 banks hold the x'-preload (chunks of 4 steps, double buffered by
    step parity), 4 banks are out-accumulators (stream x parity ping-pong).
"""

import numpy as np

import concourse.bacc as bacc
import concourse.mybir as mybir
from concourse.tile import TileContext, add_dep_helper
from concourse import bass_utils

F16 = mybir.dt.float16
F32 = mybir.dt.float32

B, T_FULL, I, H, O = 64, 1024, 64, 512, 64
NCORES = 8
NST = 2                    # streams (time segments) per core
SEG = NCORES * NST         # 16 segments
SOUT = T_FULL // SEG       # 64 output steps per segment
W = 40                     # warmup steps per segment
TL = W + SOUT              # 112 local steps per stream
KT = H // 128              # 4 tiles over H
CH = 4                     # steps per psum refill chunk
NCH = TL // CH             # 28 chunks
NX = 3                     # extra x rows: brec, u=h0@wrec.T, a=arctanh(h0)
IR = I + NX                # 67 rhs rows for the x-projection
GB0 = W - 16               # first step entering the output EMA

# packed-weights column offsets (fp16, [128, WPK])
WT_OFF = 0                 # 4 k-tiles x 512
HI_OFF = 2048              # 0.5*I, 128
WO_OFF = 2176              # 4 k-tiles x 128 (wo/2 zero-padded)
WI_OFF = 2688              # wiA on rows 0:67, 512
WPK = 3200


def build():
    nc = bacc.Bacc("TRN2", target_bir_lowering=False, debug=False)
    pe_prev = [None]

    def mm(*args, **kw):
        inst = nc.tensor.matmul(*args, **kw)
        if pe_prev[0] is not None:
            add_dep_helper(inst.ins, pe_prev[0].ins, sync=False, reason="pe order")
        pe_prev[0] = inst
        return inst

    d_wpk = nc.dram_tensor("wpk", [128, WPK], F16, kind="ExternalInput")
    d_xT = nc.dram_tensor("xT", [IR, NST * TL * 64], F16, kind="ExternalInput")
    d_out = nc.dram_tensor("outT", [O, NST * SOUT * 64], F16, kind="ExternalOutput")

    with TileContext(nc) as tc:
        with (
            tc.tile_pool(name="wpool", bufs=1) as wpool,
            tc.tile_pool(name="rz", bufs=1) as rzpool,
            tc.tile_pool(name="px", bufs=1, space="PSUM") as px,
        ):
            wpk = wpool.tile([128, WPK], F16, tag="wpk")
            nc.sync.dma_start(wpk[:], d_wpk[:])

            xT = wpool.tile([IR, NST * TL * 64], F16, tag="xT")
            CB = NST * CH * 64                       # cols per chunk = 512
            # x pieces go on the ACT DMA queue so they overlap the wpk DMA
            for lo, hi in ((0, 4 * CB), (4 * CB, 16 * CB), (16 * CB, NCH * CB)):
                nc.scalar.dma_start(xT[:, lo:hi], d_xT[:, lo:hi])

            wT = [wpk[:, WT_OFF + k * 512 : WT_OFF + (k + 1) * 512] for k in range(KT)]
            hI = wpk[:, HI_OFF : HI_OFF + 128]
            hI64 = wpk[0:64, HI_OFF : HI_OFF + 128]
            wo = [wpk[:, WO_OFF + k * 128 : WO_OFF + (k + 1) * 128] for k in range(KT)]
            wi = wpk[:, WI_OFF : WI_OFF + 512]

            r_t = [[rzpool.tile([128, KT * 64], F16, tag=f"r{st}{p}", name=f"r{st}{p}")
                    for p in range(2)] for st in range(NST)]
            zsb = [[rzpool.tile([128, KT * 64], F16, tag=f"z{st}{p}", name=f"z{st}{p}")
                    for p in range(2)] for st in range(NST)]
            # fp16 staging for the output EMA (+ scratch cols for warmup steps)
            ostg = [wpool.tile([64, SOUT * 64 + 128], F16, tag=f"os{st}", name=f"os{st}")
                    for st in range(NST)]

            # psum: 4 x-banks [4m x 2q x 64b] + 4 out-accumulator banks
            bank = [[px.tile([128, 512], F32, tag=f"px{st}{p}", name=f"px{st}{p}")
                     for p in range(2)] for st in range(NST)]
            obank = [[px.tile([128, 512], F32, tag=f"po{st}{p}", name=f"po{st}{p}")
                      for p in range(2)] for st in range(NST)]

            xTr = xT.rearrange(
                "p (c s par q b) -> p c s par q b",
                c=NCH, s=NST, par=2, q=CH // 2, b=64,
            )

            def refill(st, c, par):
                rhs = xTr[:, c, st, par, :, :]          # [IR, 2, 64] contiguous
                for m in range(KT):
                    mm(
                        bank[st][par][:, m * 128 : (m + 1) * 128],
                        lhsT=wi[:IR, m * 128 : (m + 1) * 128],
                        rhs=rhs,
                        start=(m == 0),
                        stop=False,
                        skip_group_check=True,
                    )

            def ostg_ap(st, t):
                """fp16 staging slot for out_t (scratch cols during warmup)."""
                if t >= W:
                    return ostg[st][:, (t - W) * 64 : (t - W + 1) * 64]
                return ostg[st][:, SOUT * 64 + (t % 2) * 64 : SOUT * 64 + (t % 2 + 1) * 64]

            for st in range(NST):
                refill(st, 0, 0)
                refill(st, 0, 1)

            for t in range(TL):
                c, tt = divmod(t, CH)
                par, q = tt % 2, tt // 2
                for st in range(NST):
                    if tt == 0 and c > 0:
                        refill(st, c, 0)
                    if tt == 1 and c > 0:
                        refill(st, c, 1)
                    bq = bank[st][par].rearrange("p (m c) -> p m c", c=128)[
                        :, :, q * 64 : (q + 1) * 64
                    ]                                     # [128, 4m, 64] this step
                    # skewed output EMA for step t-1, FIRST in the slot (its
                    # inputs are a full slot old, so it absorbs chain latency
                    # and its psum result is ready early for the out-CAST):
                    # out_{t-1} = 0.5 out_{t-2} + r_{t-1} @ (wo/2)
                    tp = t - 1
                    if tp >= GB0:
                        po = obank[st][tp % 2][:, 0:64]
                        if tp > GB0:
                            mm(po, lhsT=hI64, rhs=ostg_ap(st, tp - 1),
                               start=True, stop=False, skip_group_check=True)
                        for k in range(KT):
                            mm(po, lhsT=wo[k],
                               rhs=r_t[st][1 - par][:, k * 64 : (k + 1) * 64],
                               start=(k == 0 and tp == GB0),
                               stop=(k == KT - 1),
                               skip_group_check=True)
                        nc.vector.tensor_copy(ostg_ap(st, tp), po[:O, :])
                        u = tp - W
                        if tp >= W and u % 8 == 7:        # flush 8 finished cols
                            nc.sync.dma_start(
                                d_out[:, st * SOUT * 64 + (u - 7) * 64 :
                                      st * SOUT * 64 + (u + 1) * 64],
                                ostg[st][:, (u - 7) * 64 : (u + 1) * 64],
                            )
                    if t > 0:
                        mm(                               # inject 0.5*z_{t-1}
                            bq[:],
                            lhsT=hI,
                            rhs=zsb[st][1 - par].rearrange("p (m b) -> p m b", b=64)[:],
                            start=False,
                            stop=False,
                            skip_group_check=True,
                        )
                        for k in range(KT):
                            for m in range(KT):
                                mm(
                                    bank[st][par][:, m * 128 + q * 64 : m * 128 + (q + 1) * 64],
                                    lhsT=wT[k][:, m * 128 : (m + 1) * 128],
                                    rhs=r_t[st][1 - par][:, k * 64 : (k + 1) * 64],
                                    start=False,
                                    stop=False,
                                    skip_group_check=True,
                                )
                    rv = r_t[st][par].rearrange("p (m b) -> p m b", b=64)
                    nc.vector.tensor_copy(
                        zsb[st][par].rearrange("p (m b) -> p m b", b=64)[:], bq
                    )
                    nc.scalar.activation(rv[:], bq, mybir.ActivationFunctionType.Tanh)

            # drain: output EMA for the final step of each stream
            for st in range(NST):
                tp = TL - 1
                par = tp % 2                              # parity of step tp
                po = obank[st][tp % 2][:, 0:64]
                mm(po, lhsT=hI64, rhs=ostg_ap(st, tp - 1),
                   start=True, stop=False, skip_group_check=True)
                for k in range(KT):
                    mm(po, lhsT=wo[k], rhs=r_t[st][par][:, k * 64 : (k + 1) * 64],
                       start=False, stop=(k == KT - 1), skip_group_check=True)
                nc.vector.tensor_copy(ostg_ap(st, tp), po[:O, :])
                u = tp - W
                nc.sync.dma_start(
                    d_out[:, st * SOUT * 64 + (u - 7) * 64 : st * SOUT * 64 + (u + 1) * 64],
                    ostg[st][:, (u - 7) * 64 : (u + 1) * 64],
                )

    _thin_pe_clock(nc)
    nc.compile()
    return nc


def _thin_pe_clock(nc):
    """Strip unreferenced PE engine-clock increments from the BIR.

    Tile attaches a sem-inc to EVERY matmul; the semaphore-update pipeline
    sustains only ~34ns/inc, so the inc stream (not the PE) can become the
    clock.  Keeping increments only at ticks some wait references (and
    remapping waits to their rank) is semantically equivalent.
    """
    import bisect

    fn = nc.m.functions[0]
    SEM = None
    for blk in fn.blocks:
        for inst in blk.instructions:
            si = inst.sync_info
            if si is None:
                continue
            for u in si.on_update:
                if u.ant_name and u.ant_name.startswith("PE_") and u.update_mode == "sem-inc":
                    SEM = u.id
                    break
            if SEM is not None:
                break
        if SEM is not None:
            break
    if SEM is None:
        return
    refs = set()
    for blk in fn.blocks:
        for inst in blk.instructions:
            si = inst.sync_info
            if si is None:
                continue
            for w in si.on_wait:
                if w.id == SEM:
                    assert w.wait_mode == "sem-ge-imm", w.wait_mode
                    refs.add(w.wait_value)
    kept = sorted(refs)
    tick = 0
    for blk in fn.blocks:
        for inst in blk.instructions:
            si = inst.sync_info
            if si is None:
                continue
            ups = list(si.on_update)
            has = [u for u in ups if u.id == SEM]
            if has:
                assert len(has) == 1 and has[0].update_value == 1
                tick += 1
                if tick not in refs:
                    si.on_update = [u for u in ups if u.id != SEM]
    for blk in fn.blocks:
        for inst in blk.instructions:
            si = inst.sync_info
            if si is None:
                continue
            for w in si.on_wait:
                if w.id == SEM:
                    w.wait_value = bisect.bisect_right(kept, w.wait_value)


_CACHE = {}


def _get_nc():
    if "nc" not in _CACHE:
        _CACHE["nc"] = build()
    return _CACHE["nc"]


def prep_inputs(input, wi, wrec, wo, brec, h0):
    """Host-side layout prep. Returns list of 8 in_maps (xT differs per core)."""
    input = np.asarray(input, dtype=np.float32)
    wi = np.asarray(wi, dtype=np.float32)
    wrec = np.asarray(wrec, dtype=np.float32)
    wo = np.asarray(wo, dtype=np.float32)
    brec = np.asarray(brec, dtype=np.float32)
    h0 = np.asarray(h0, dtype=np.float32)

    wTh = (0.5 * wrec.T).astype(np.float16)
    h0c = np.clip(h0, -1 + 1e-6, 1 - 1e-6)
    a_vec = np.arctanh(h0c).astype(np.float32)
    u_vec = 2.0 * (h0c @ wTh.astype(np.float32))     # h0 @ wrec.T (quantized)
    wiA = np.concatenate(
        [wi, brec[None, :], u_vec[None, :], a_vec[None, :]], axis=0
    ).astype(np.float16)

    wpk = np.zeros((128, WPK), np.float16)
    for k in range(KT):
        wpk[:, WT_OFF + k * 512 : WT_OFF + (k + 1) * 512] = wTh[k * 128 : (k + 1) * 128]
    wpk[:, HI_OFF : HI_OFF + 128] = (0.5 * np.eye(128)).astype(np.float16)
    woh = (wo / 2.0).astype(np.float16)
    for k in range(KT):
        wpk[:, WO_OFF + k * 128 : WO_OFF + k * 128 + 64] = woh[k * 128 : (k + 1) * 128]
    wpk[:IR, WI_OFF : WI_OFF + 512] = wiA

    x16 = input.astype(np.float16).astype(np.float32)

    in_maps = []
    for core in range(NCORES):
        xA = np.zeros((IR, NST, TL, 64), np.float32)
        for st in range(NST):
            s = NST * core + st
            t0 = s * SOUT
            for j in range(TL):
                g = t0 - W + j
                if s == 0:
                    if j == 0:
                        xA[I + 2, st, j] = 1.0                       # z0 = arctanh(h0)
                    elif j < W:
                        xA[I + 1, st, j] = -0.5                      # hold z at z*
                        xA[I + 2, st, j] = 0.5
                    elif j == W:
                        xA[:I, st, j] = x16[:, 0].T                  # onto true x_0
                        xA[I, st, j] = 1.0
                        xA[I + 1, st, j] = 0.5
                        xA[I + 2, st, j] = -0.5
                    else:
                        xA[:I, st, j] = (x16[:, g] - 0.5 * x16[:, g - 1]).T
                        xA[I, st, j] = 0.5
                else:
                    if j == 0:
                        xA[:I, st, j] = x16[:, g].T                  # z0 = h0 wrec.T + x
                        xA[I, st, j] = 1.0
                        xA[I + 1, st, j] = 1.0
                    else:
                        xA[:I, st, j] = (x16[:, g] - 0.5 * x16[:, g - 1]).T
                        xA[I, st, j] = 0.5
        # chunk-major reorder: [st, (c,q,par)] -> [c, st, par, q]
        xA = xA.reshape(IR, NST, NCH, CH // 2, 2, 64).transpose(0, 2, 1, 4, 3, 5)
        xA = np.ascontiguousarray(xA).reshape(IR, NST * TL * 64).astype(np.float16)
        in_maps.append({"wpk": wpk, "xT": xA})
    return in_maps


def run_sharded(inputs, t_steps=T_FULL, trace=False):
    assert t_steps == T_FULL, "kernel is built for the full 1024 steps"
    nc = _get_nc()
    in_maps = prep_inputs(**inputs)
    res = bass_utils.run_bass_kernel_spmd(
        nc, in_maps, core_ids=list(range(NCORES)), trace=trace
    )
    out = np.empty((B, T_FULL, O), np.float32)
    for core in range(NCORES):
        oT = res.results[core]["outT"].astype(np.float32)  # [O, NST*SOUT*64]
        for st in range(NST):
            s = NST * core + st
            blk = oT[:, st * SOUT * 64 : (st + 1) * SOUT * 64].reshape(O, SOUT, 64)
            out[:, s * SOUT : (s + 1) * SOUT] = np.transpose(blk, (2, 1, 0))
    return out, res


def kernel(input, wi, wrec, wo, brec, h0):
    out, _ = run_sharded(
        dict(input=input, wi=wi, wrec=wrec, wo=wo, brec=brec, h0=h0),
        t_steps=T_FULL,
        trace=False,
    )
    return out
